# revision 1
# baseline (speedup 1.0000x reference)
"""Trainium2 Bass kernel for nn_BloodhoundSub_12463995093069.

2-layer decoder with broadcast cross-attention -> cosine similarity [8, 32].

Sharding: candidates (BC=32) split 4 per core across 8 cores. Each core runs
the full decoder for its 4 candidates against all 8 query batches; the host
concatenates the per-core [8, 4] outputs along axis 1.

v2: fp8 DoubleRow matmuls for all D-contraction projections (QKV/O for both
attention types, FFN w1) at 4x fewer PE cycles than the bf16 path; FFN w2 as
fp8 hi+lo with a DoubleRow correction pass (bf16-level accuracy at half the
bf16 PE cost). Residual stream in bf16; attention internals bf16. V-bias
folded into the O-projection bias (A rows sum to 1). All weights resident in
SBUF from kernel start; host pre-transposes every tensor into its SBUF layout
so DMAs are contiguous.
"""

import sys

if "/opt/trn_rl_repo" not in sys.path:
    sys.path.insert(0, "/opt/trn_rl_repo")

import numpy as np
from contextlib import ExitStack

# ---- dims ----
L = 2
D = 512
N = 8
H = 64
FF = 2048
F = 256
BQ = 8
BC = 32
TQ = 128
TC = 128
EPS = 1e-6
SCALE = 1.0 / 8.0  # 1/sqrt(H)

NCORES = 8
BCC = BC // NCORES
P = 128
KC = D // P     # 4 feature chunks
MB = D // P     # 4 output chunks
FFC = FF // P   # 16
T1 = BCC * TC   # 512
TB = 512        # tokens per block
NBLK = BQ
T = NBLK * TB   # 4096
TQALL = BQ * TQ  # 1024

# fp8 scales (input_scale * weight_scale == 1 so psum is unscaled)
SX = 0.125      # x cast scale
SW = 8.0        # qkv/w1 weight scale
SO = 0.25       # o cast scale (folded into recip)
SWO = 4.0       # wo weight scale
SH = 0.25       # h cast scale
SW2 = 4.0       # w2 weight scale
WLO = 64.0      # w2 lo-residual gain

_BUILT = None


def build_program():
    import concourse.bass as bass
    import concourse.tile as tile
    import concourse.mybir as mybir
    from concourse import bacc

    F32 = mybir.dt.float32
    F32R = mybir.dt.float32r
    BF16 = mybir.dt.bfloat16
    F8 = mybir.dt.float8e4

    nc = bacc.Bacc("TRN2", target_bir_lowering=False, debug=False)
    tens = {}

    def din(name, shape, dt):
        tens[name] = nc.dram_tensor(name, shape, dt, kind="ExternalInput")

    din("x0", [P, KC, T1], BF16)
    din("q_bf", [P, KC, TQALL], BF16)
    din("q8", [P, KC, TQALL], F8)
    for l in range(L):
        for pfx in ("sa", "ca"):
            din(f"{pfx}_wq8_{l}", [P, KC, D], F8)
            din(f"{pfx}_wk8_{l}", [P, KC, D], F8)
            din(f"{pfx}_wv8_{l}", [P, KC, D], F8)
            din(f"{pfx}_wo8_{l}", [P, KC, D], F8)
            din(f"{pfx}_bq_{l}", [P, MB], F32)
            din(f"{pfx}_bk_{l}", [P, MB], F32)
            din(f"{pfx}_bo_{l}", [P, MB], F32)
        din(f"ffn_w1_{l}", [P, KC, FF], F8)
        din(f"ffn_w2_{l}", [P, FFC, D], BF16)
    din("feat_wq", [P, KC, F], F32R)
    din("feat_wc", [P, KC, F], F32R)
    din("colsel", [P, 8, 8], F32R)
    din("colsel_bf", [P, 8, 8], BF16)
    din("densel_bf", [P, 8, 8], BF16)
    din("rowsel_bf", [8, 8, P], BF16)
    din("selpair_bf", [8, 4, P], BF16)
    tens["out"] = nc.dram_tensor("out", [1, BQ * BCC], F32, kind="ExternalOutput")

    with tile.TileContext(nc) as tc, ExitStack() as ctx:
        with nc.allow_low_precision(reason="bf16/fp8 matmul pipeline"):
            _emit(nc, tc, ctx, tens)
    nc.compile()
    return nc


def _emit(nc, tc, ctx, tens):
    import concourse.mybir as mybir

    F32 = mybir.dt.float32
    F32R = mybir.dt.float32r
    BF16 = mybir.dt.bfloat16
    F8 = mybir.dt.float8e4
    AF = mybir.ActivationFunctionType
    ALU = mybir.AluOpType
    DR = mybir.MatmulPerfMode.DoubleRow

    def r(ap):
        return ap.bitcast(F32R)

    # ---------------- pools ----------------
    const = ctx.enter_context(tc.tile_pool(name="const", bufs=1))
    wpool = ctx.enter_context(tc.tile_pool(name="wpool", bufs=1))
    xpool = ctx.enter_context(tc.tile_pool(name="xpool", bufs=1))
    stats_ch = ctx.enter_context(tc.tile_pool(name="stats_ch", bufs=1))
    ps = ctx.enter_context(tc.tile_pool(name="ps", bufs=3, space="PSUM"))
    ps_attn = ctx.enter_context(tc.tile_pool(name="ps_attn", bufs=2, space="PSUM"))
    ps_small = ctx.enter_context(tc.tile_pool(name="ps_small", bufs=1, space="PSUM"))

    # ---------------- constants ----------------
    eps_t = const.tile([P, 1], F32)
    nc.vector.memset(eps_t[:], EPS)
    colsel = const.tile([P, 8, 8], F32R)
    nc.sync.dma_start(colsel[:], tens["colsel"][:])
    colsel_bf = const.tile([P, 8, 8], BF16)
    nc.sync.dma_start(colsel_bf[:], tens["colsel_bf"][:])
    densel_bf = const.tile([P, 8, 8], BF16)
    nc.sync.dma_start(densel_bf[:], tens["densel_bf"][:])
    rowsel_bf = const.tile([8, 8, P], BF16)
    nc.sync.dma_start(rowsel_bf[:], tens["rowsel_bf"][:])
    selpair_bf = const.tile([8, 4, P], BF16)
    nc.sync.dma_start(selpair_bf[:], tens["selpair_bf"][:])

    # ---------------- persistent activations (loaded first) ----------------
    x_t = xpool.tile([P, KC, T], BF16)       # main residual (from CA0 onward)
    x0_t = xpool.tile([P, KC, T1], BF16)     # layer-0 SA/CA input
    nc.sync.dma_start(x0_t[:], tens["x0"][:])
    q8 = xpool.tile([P, KC, TQALL], F8)      # q memory fp8 (CA K/V input)
    nc.sync.dma_start(q8[:], tens["q8"][:])
    qp = xpool.tile([P, KC, BQ], F32R)       # pooled q (token mean * 127)
    cp = xpool.tile([P, KC, BQ * BCC], F32R)  # pooled x (token mean * 127)
    with tc.tile_pool(name="qpool", bufs=1) as qpl:
        q_bf = qpl.tile([P, KC, TQALL], BF16)  # q memory (pooling only)
        nc.sync.dma_start(q_bf[:], tens["q_bf"][:])
        for k in range(KC):
            nc.vector.tensor_reduce(
                qp[:, k, :],
                q_bf[:, k, :].rearrange("p (e t) -> p e t", e=BQ)[:, :, 1:],
                mybir.AxisListType.X, ALU.add,
            )

    # ---------------- persistent weights ----------------
    W = {}
    for l in range(L):
        for pfx in ("sa", "ca"):
            for wn in ("wq8", "wk8", "wv8", "wo8"):
                t = wpool.tile([P, KC, D], F8, tag=f"{pfx}_{wn}_{l}")
                nc.sync.dma_start(t[:], tens[f"{pfx}_{wn}_{l}"][:])
                W[f"{pfx}_{wn}_{l}"] = t
            for bn in ("bq", "bk", "bo"):
                t = wpool.tile([P, MB], F32, tag=f"{pfx}_{bn}_{l}")
                nc.sync.dma_start(t[:], tens[f"{pfx}_{bn}_{l}"][:])
                W[f"{pfx}_{bn}_{l}"] = t
        t = wpool.tile([P, KC, FF], F8, tag=f"w1_{l}")
        nc.sync.dma_start(t[:], tens[f"ffn_w1_{l}"][:])
        W[f"w1_{l}"] = t
        t = wpool.tile([P, FFC, D], BF16, tag=f"w2_{l}")
        nc.sync.dma_start(t[:], tens[f"ffn_w2_{l}"][:])
        W[f"w2_{l}"] = t

    # ============ helpers ============

    def cast_x8(pool, x_of):
        """fp8 cast of 4 chunks of x (scale SX)."""
        x8 = pool.tile([P, KC, TB], F8, tag="x8")
        for k in range(KC):
            nc.scalar.activation(x8[:, k, :], x_of(k), AF.Identity, scale=SX)
        return x8

    def proj_dr(w_t, x8_of, out_of, bias_t=None, nT=TB):
        """Feature-major DR projection.

        x8_of(j) -> [P, 2, nT] fp8 AP for chunk pair j.
        out[mb] = sum_j w[:, 2j:2j+2, mbcols].T (x) x8_of(j)  (+bias).
        """
        for mb_i in range(MB):
            acc = ps.tile([P, TB], F32, tag="gemm")
            for j in range(KC // 2):
                nc.tensor.matmul(
                    acc[:, :nT],
                    w_t[:, 2 * j : 2 * j + 2, mb_i * P : (mb_i + 1) * P],
                    x8_of(j),
                    start=(j == 0), stop=(j == KC // 2 - 1),
                    perf_mode=DR,
                )
            if bias_t is not None:
                nc.scalar.activation(
                    out_of(mb_i), acc[:, :nT], AF.Identity,
                    bias=bias_t[:, mb_i : mb_i + 1],
                )
            else:
                nc.scalar.copy(out_of(mb_i), acc[:, :nT])

    def vproj_dr(w_t, x8_of, out_sb):
        """Token-major DR V projection for one 128-token sub-block.

        x8_of(j) -> [P, 2, 128] fp8 AP (the sub-block's tokens, chunk pair j).
        out_sb [128, D] bf16.
        """
        acc = ps.tile([P, TB], F32, tag="gemm")
        for j in range(KC // 2):
            nc.tensor.matmul(
                acc[:, :D],
                x8_of(j),
                w_t[:, 2 * j : 2 * j + 2, :],
                start=(j == 0), stop=(j == KC // 2 - 1),
                perf_mode=DR,
            )
        nc.vector.tensor_copy(out_sb, acc[:, :D])

    def pos_of(n):
        # head order: slot p < 4 holds head 2p (hs=0), slot p >= 4 holds
        # head 2(p-4)+1 (hs=64) -- a psum bank may only mix matmul groups
        # with the SAME contraction partition base.
        return n // 2 if n % 2 == 0 else 4 + n // 2

    def attn_front(pool, q_sb, kv_of, nsub):
        """Scores + exp + softmax denominators for one 512-token block.

        Returns (e_all, recip): e_all [P, nsub, N(slots), twid] bf16,
        recip [8, TB] bf16 scaled by SO.
        """
        twid = TB // nsub
        hpb = min(4, TB // twid)  # heads packed per score psum tile
        den_ps = ps_small.tile([8, TB], F32, tag="den")
        e_all = pool.tile([P, nsub, N, twid], BF16, tag="exp")
        for sub in range(nsub):
            k_of, _ = kv_of(sub)
            for ng in range(N // hpb):
                s_ps = ps_attn.tile([P, TB], F32, tag="attn")
                for ni in range(hpb):
                    p_slot = ng * hpb + ni
                    n = 2 * p_slot if p_slot < 4 else 2 * (p_slot - 4) + 1
                    hs = (n % 2) * H
                    nc.tensor.matmul(
                        s_ps[:, ni * twid : (ni + 1) * twid],
                        k_of(n),
                        q_sb[hs : hs + H, n // 2, sub * twid : (sub + 1) * twid],
                        start=True, stop=True,
                    )
                nc.scalar.activation(
                    e_all[:, sub, ng * hpb : (ng + 1) * hpb, :]
                    .rearrange("p n t -> p (n t)"),
                    s_ps[:, : hpb * twid], AF.Exp, scale=SCALE,
                )
            for p_slot in range(N):
                nc.tensor.matmul(
                    den_ps[:, sub * twid : (sub + 1) * twid],
                    densel_bf[:, p_slot, :],
                    e_all[:, sub, p_slot, :],
                    start=(p_slot == 0), stop=(p_slot == N - 1),
                )
        recip_f = pool.tile([8, TB], F32, tag="recipf")
        nc.vector.reciprocal_approx_fast(recip_f[:], den_ps[:])
        recip = pool.tile([8, TB], BF16, tag="recip")
        nc.scalar.activation(recip[:], recip_f[:], AF.Identity, scale=SO)
        return e_all, recip

    def attn_back(pool, e_all, recip, kv_of, o8_sb, nsub):
        """rb broadcast + AV + normalize into o8_sb [P, MB, TB] fp8 (x SO)."""
        twid = TB // nsub
        for sub in range(nsub):
            _, v_of = kv_of(sub)
            if twid <= 128:
                # rb for all 4 head-pairs in one psum tile [P, 4*twid]
                rb = ps_attn.tile([P, TB], F32, tag="attn")
                for hp in range(4):
                    nc.tensor.matmul(
                        rb[:, hp * twid : (hp + 1) * twid],
                        selpair_bf[:, hp, :],
                        recip[:, sub * twid : (sub + 1) * twid],
                        start=True, stop=True,
                    )
                rb_sb = pool.tile([P, 4, twid], BF16, tag="rb", bufs=1)
                nc.scalar.copy(rb_sb[:].rearrange("p c t -> p (c t)"),
                               rb[:, : 4 * twid])
                for hg in range(2):  # 2 head-pairs per av psum tile
                    av = ps_attn.tile([P, TB], F32, tag="attn")
                    for hi in range(2):
                        hp = hg * 2 + hi
                        for j in range(2):
                            n = 2 * hp + j
                            nc.tensor.matmul(
                                av[j * H : (j + 1) * H,
                                   hi * twid : (hi + 1) * twid],
                                v_of(n),
                                e_all[:, sub, pos_of(n), :],
                                start=True, stop=True,
                                tile_position=(0, j * H),
                            )
                    nc.vector.tensor_tensor(
                        o8_sb[:, hg * 2 : hg * 2 + 2,
                              sub * twid : (sub + 1) * twid],
                        av[:, : 2 * twid].rearrange("p (c t) -> p c t", c=2),
                        rb_sb[:, hg * 2 : hg * 2 + 2, :],
                        ALU.mult,
                    )
            else:
                for hp in range(4):
                    rb = ps_attn.tile([P, TB], F32, tag="attn")
                    nc.tensor.matmul(
                        rb[:, :twid], selpair_bf[:, hp, :],
                        recip[:, sub * twid : (sub + 1) * twid],
                        start=True, stop=True,
                    )
                    rb_sb = pool.tile([P, TB], BF16, tag="rb", bufs=1)
                    nc.scalar.copy(rb_sb[:, :twid], rb[:, :twid])
                    av = ps_attn.tile([P, TB], F32, tag="attn")
                    for j in range(2):
                        n = 2 * hp + j
                        nc.tensor.matmul(
                            av[j * H : (j + 1) * H, :twid],
                            v_of(n),
                            e_all[:, sub, pos_of(n), :],
                            start=True, stop=True,
                            tile_position=(0, j * H),
                        )
                    nc.vector.tensor_tensor(
                        o8_sb[:, hp, sub * twid : (sub + 1) * twid],
                        av[:, :twid],
                        rb_sb[:, :twid],
                        ALU.mult,
                    )

    def oproj_residual(wo_t, bo_t, o8_sb, x_io_of):
        """x_io[mb] += wo.T (x) o8 + bo   (in place, bf16)."""
        for mb_i in range(MB):
            acc = ps.tile([P, TB], F32, tag="gemm")
            for j in range(KC // 2):
                nc.tensor.matmul(
                    acc[:],
                    wo_t[:, 2 * j : 2 * j + 2, mb_i * P : (mb_i + 1) * P],
                    o8_sb[:, 2 * j : 2 * j + 2, :],
                    start=(j == 0), stop=(j == KC // 2 - 1),
                    perf_mode=DR,
                )
            nc.vector.scalar_tensor_tensor(
                x_io_of(mb_i), acc[:], bo_t[:, mb_i : mb_i + 1],
                x_io_of(mb_i), ALU.add, ALU.add,
            )

    def stats_block(pool, x_of, s1_ps, s2_ps, blk, first, last):
        """Accumulate sum(x) / sum(x^2) of block blk into the stats psums.

        The stats matmuls write the full [8, TB] psum (zero rows off-target),
        so only the very first matmul of the pass may use start=True.
        """
        sq_t = pool.tile([P, KC, TB], BF16, tag="sqc")
        for k in range(KC):
            nc.gpsimd.tensor_tensor(sq_t[:, k, :], x_of(k), x_of(k), ALU.mult)
        for k in range(KC):
            nc.tensor.matmul(s1_ps[:], colsel_bf[:, blk, :], x_of(k),
                             start=(first and k == 0),
                             stop=(last and k == KC - 1))
            nc.tensor.matmul(s2_ps[:], colsel_bf[:, blk, :], sq_t[:, k, :],
                             start=(first and k == 0),
                             stop=(last and k == KC - 1))

    def ln_chain(s1_ps, s2_ps, nblk):
        """Turn stats psums into LN scale a / offset c (bf16, SBUF).

        Reuses the stats psum banks as scratch (everything on rows 0-7):
          s1_ps <- m = s1/D ; s2_ps <- sd = sqrt(s2/D - m^2 + eps).
        """
        # colsel_bf carries 1/D, so s1 = m and s2 = E[x^2] directly
        u = stats_ch.tile([8, TB], F32, tag="ln_u")
        u2 = stats_ch.tile([8, TB], F32, tag="ln_u2")
        ac = stats_ch.tile([8, 2, TB], BF16, tag="ln_ac")
        nc.scalar.activation(u[:nblk], s1_ps[:nblk], AF.Square)  # m^2
        nc.vector.tensor_tensor(u[:nblk], s2_ps[:nblk], u[:nblk], ALU.subtract)
        nc.scalar.activation(u[:nblk], u[:nblk], AF.Sqrt, bias=eps_t[:nblk, :])
        nc.vector.reciprocal_approx_fast(u2[:nblk], u[:nblk])
        a_sb = ac[:, 0, :]
        nc.scalar.copy(a_sb[:nblk], u2[:nblk])
        c_sb = ac[:, 1, :]
        nc.vector.tensor_tensor(c_sb[:nblk], s1_ps[:nblk], u2[:nblk],
                                ALU.mult)
        return a_sb, c_sb

    def ln_apply(pool, a_sb, c_sb, blk, x_of, nblk=NBLK):
        a_ps = ps.tile([P, TB], F32, tag="gemm")
        nc.tensor.matmul(a_ps[:], rowsel_bf[:nblk, blk, :], a_sb[:nblk, :],
                         start=True, stop=True)
        c_ps = ps.tile([P, TB], F32, tag="gemm")
        nc.tensor.matmul(c_ps[:], rowsel_bf[:nblk, blk, :], c_sb[:nblk, :],
                         start=True, stop=True)
        ab = pool.tile([P, 2, TB], BF16, tag="lnbc")
        nc.scalar.copy(ab[:, 0, :], a_ps[:])
        nc.scalar.copy(ab[:, 1, :], c_ps[:])
        for mb_i in range(MB):
            tmp = pool.tile([P, TB], BF16, tag="lntmp")
            nc.vector.tensor_tensor(tmp[:], x_of(mb_i), ab[:, 0, :], ALU.mult)
            nc.vector.tensor_tensor(x_of(mb_i), tmp[:], ab[:, 1, :],
                                    ALU.subtract)

    pending_ln = [None]  # (a_sb, c_sb, x_of(m, blk), nblk, done:set)

    def apply_ln_upto(pool, hi):
        st = pending_ln[0]
        if st is None:
            return
        a_sb, c_sb, x_of, nprev, done = st
        for b in range(min(hi + 1, nprev)):
            if b in done:
                continue
            ln_apply(pool, a_sb, c_sb, b, lambda m, b=b: x_of(m, b),
                     nblk=nprev)
            done.add(b)
        if len(done) == nprev:
            pending_ln[0] = None

    # =========================================================
    import os
    npass = int(os.environ.get("BASS_NPASS", "99"))
    if npass < 99:
        nc.vector.memset(x_t[:], 0.0)
    pcount = 0
    for l in range(L):
        pcount += 1
        if pcount > npass:
            break
        # ---------------- SA pass ----------------
        with ExitStack() as sctx:
            tp = sctx.enter_context(tc.tile_pool(name=f"sat{l}", bufs=2))
            wq = W[f"sa_wq8_{l}"]; wk = W[f"sa_wk8_{l}"]
            wv = W[f"sa_wv8_{l}"]; wo = W[f"sa_wo8_{l}"]
            bq = W[f"sa_bq_{l}"]; bk = W[f"sa_bk_{l}"]; bo = W[f"sa_bo_{l}"]
            s1_ps = ps_small.tile([8, TB], F32, tag="s1")
            s2_ps = ps_small.tile([8, TB], F32, tag="s2")

            nblk = 1 if l == 0 else NBLK

            def xin_ap(k, blk):
                if l == 0:
                    return x0_t[:, k, :]
                return x_t[:, k, blk * TB : (blk + 1) * TB]

            def sa_stage1(blk):
                x8 = cast_x8(tp, lambda k: xin_ap(k, blk))
                q_sb = tp.tile([P, KC, TB], BF16, tag="q")
                k_sb = tp.tile([P, KC, TB], BF16, tag="k")
                v_sb = tp.tile([P, BCC, D], BF16, tag="v")
                proj_dr(wq, lambda j: x8[:, 2 * j : 2 * j + 2, :],
                        lambda m: q_sb[:, m, :], bias_t=bq)
                proj_dr(wk, lambda j: x8[:, 2 * j : 2 * j + 2, :],
                        lambda m: k_sb[:, m, :], bias_t=bk)
                for sub in range(BCC):
                    vproj_dr(wv,
                             lambda j, sub=sub: x8[
                                 :, 2 * j : 2 * j + 2, sub * P : (sub + 1) * P],
                             v_sb[:, sub, :])

                def kv_of(sub):
                    def k_of(n):
                        hs = (n % 2) * H
                        return k_sb[hs : hs + H, n // 2, sub * P : (sub + 1) * P]

                    def v_of(n):
                        return v_sb[:, sub, n * H : (n + 1) * H]

                    return k_of, v_of

                e_all, recip = attn_front(tp, q_sb, kv_of, BCC)
                return blk, kv_of, e_all, recip

            def sa_stage2(st):
                blk, kv_of, e_all, recip = st
                o8_sb = tp.tile([P, MB, TB], F8, tag="o")
                attn_back(tp, e_all, recip, kv_of, o8_sb, BCC)
                oproj_residual(wo, bo, o8_sb, lambda m: xin_ap(m, blk))

            def sa_stage3(blk):
                stats_block(tp, lambda k: xin_ap(k, blk),
                            s1_ps, s2_ps, blk, blk == 0, blk == nblk - 1)

            pipe = []
            for blk in range(nblk):
                apply_ln_upto(tp, blk + 1)
                pipe.append(sa_stage1(blk))
                if len(pipe) >= 2:
                    sa_stage2(pipe[-2])
                if len(pipe) >= 3:
                    sa_stage3(pipe[-3][0])
            sa_stage2(pipe[-1])
            for blk in range(max(0, nblk - 2), nblk):
                sa_stage3(blk)
            a_sb, c_sb = ln_chain(s1_ps, s2_ps, nblk)
            pending_ln[0] = (a_sb, c_sb,
                             lambda m, blk: xin_ap(m, blk), nblk, set())

        # ---------------- CA pass ----------------
        pcount += 1
        if pcount > npass:
            break
        with ExitStack() as sctx:
            wp = sctx.enter_context(tc.tile_pool(name=f"caw{l}", bufs=1))
            tp = sctx.enter_context(tc.tile_pool(name=f"cat{l}", bufs=2))
            wq = W[f"ca_wq8_{l}"]; wk = W[f"ca_wk8_{l}"]
            wv = W[f"ca_wv8_{l}"]; wo = W[f"ca_wo8_{l}"]
            bq = W[f"ca_bq_{l}"]; bk = W[f"ca_bk_{l}"]; bo = W[f"ca_bo_{l}"]
            s1_ps = ps_small.tile([8, TB], F32, tag="s1")
            s2_ps = ps_small.tile([8, TB], F32, tag="s2")

            # K_ca^T [P, KC, TQALL] bf16 ; V_ca [P, BQ, D] bf16 (token-major)
            kca = wp.tile([P, KC, TQALL], BF16)
            for th in range(2):
                proj_dr(wk,
                        lambda j, th=th: q8[:, 2 * j : 2 * j + 2,
                                            th * TB : (th + 1) * TB],
                        lambda m, th=th: kca[:, m, th * TB : (th + 1) * TB],
                        bias_t=bk)
            vca = wp.tile([P, BQ, D], BF16)
            for e in range(BQ):
                vproj_dr(wv,
                         lambda j, e=e: q8[:, 2 * j : 2 * j + 2,
                                           e * P : (e + 1) * P],
                         vca[:, e, :])

            # L0: Q from x0 (e-independent) computed once
            if l == 0:
                apply_ln_upto(wp, 0)  # SA0's LN on x0
                x8s = cast_x8(wp, lambda k: x0_t[:, k, :])
                q_sh = wp.tile([P, KC, TB], BF16, tag="q")
                proj_dr(wq, lambda j: x8s[:, 2 * j : 2 * j + 2, :],
                        lambda m: q_sh[:, m, :], bias_t=bq)

            def ca_kv_of(e):
                def inner(sub):
                    def k_of(n):
                        hs = (n % 2) * H
                        return kca[hs : hs + H, n // 2, e * P : (e + 1) * P]

                    def v_of(n):
                        return vca[:, e, n * H : (n + 1) * H]

                    return k_of, v_of
                return inner

            def ca_stage1(e):
                if l == 0:
                    q_sb = q_sh
                else:
                    x8 = cast_x8(tp,
                                 lambda k: x_t[:, k, e * TB : (e + 1) * TB])
                    q_sb = tp.tile([P, KC, TB], BF16, tag="q2")
                    proj_dr(wq, lambda j: x8[:, 2 * j : 2 * j + 2, :],
                            lambda m: q_sb[:, m, :], bias_t=bq)
                kv_of = ca_kv_of(e)
                e_all, recip = attn_front(tp, q_sb, kv_of, 1)
                return e, kv_of, e_all, recip

            def ca_stage2(st):
                e, kv_of, e_all, recip = st
                o8_sb = tp.tile([P, MB, TB], F8, tag="o")
                attn_back(tp, e_all, recip, kv_of, o8_sb, 1)
                # residual source: x0 (l=0, broadcast) or x_t (l=1, in place)
                if l == 0:
                    for mb_i in range(MB):
                        acc = ps.tile([P, TB], F32, tag="gemm")
                        for j in range(KC // 2):
                            nc.tensor.matmul(
                                acc[:],
                                wo[:, 2 * j : 2 * j + 2,
                                   mb_i * P : (mb_i + 1) * P],
                                o8_sb[:, 2 * j : 2 * j + 2, :],
                                start=(j == 0), stop=(j == KC // 2 - 1),
                                perf_mode=DR,
                            )
                        nc.vector.scalar_tensor_tensor(
                            x_t[:, mb_i, e * TB : (e + 1) * TB],
                            acc[:], bo[:, mb_i : mb_i + 1],
                            x0_t[:, mb_i, :], ALU.add, ALU.add,
                        )
                else:
                    oproj_residual(wo, bo, o8_sb,
                                   lambda m: x_t[:, m, e * TB : (e + 1) * TB])

            def ca_stage3(e):
                stats_block(tp,
                            lambda k: x_t[:, k, e * TB : (e + 1) * TB],
                            s1_ps, s2_ps, e, e == 0, e == NBLK - 1)

            pipe = []
            for e in range(NBLK):
                if l == 1:
                    apply_ln_upto(tp, e + 1)
                pipe.append(ca_stage1(e))
                if len(pipe) >= 2:
                    ca_stage2(pipe[-2])
                if len(pipe) >= 3:
                    ca_stage3(pipe[-3][0])
            ca_stage2(pipe[-1])
            for e in range(NBLK - 2, NBLK):
                ca_stage3(e)
            a_sb, c_sb = ln_chain(s1_ps, s2_ps, NBLK)
            pending_ln[0] = (
                a_sb, c_sb,
                lambda m, blk: x_t[:, m, blk * TB : (blk + 1) * TB],
                NBLK, set())

        # ---------------- FFN pass ----------------
        pcount += 1
        if pcount > npass:
            break
        with ExitStack() as sctx:
            tp = sctx.enter_context(tc.tile_pool(name=f"ft{l}", bufs=2))
            hp2 = sctx.enter_context(tc.tile_pool(name=f"fh{l}", bufs=2))
            w1 = W[f"w1_{l}"]
            w2 = W[f"w2_{l}"]
            s1_ps = ps_small.tile([8, TB], F32, tag="s1")
            s2_ps = ps_small.tile([8, TB], F32, tag="s2")

            def ffn_w1(blk):
                x8 = cast_x8(tp,
                             lambda k: x_t[:, k, blk * TB : (blk + 1) * TB])
                h_sb = hp2.tile([P, FFC, TB], BF16, tag="h")
                for mf in range(FFC):
                    acc = ps_attn.tile([P, TB], F32, tag="attn")
                    for j in range(KC // 2):
                        nc.tensor.matmul(
                            acc[:],
                            w1[:, 2 * j : 2 * j + 2, mf * P : (mf + 1) * P],
                            x8[:, 2 * j : 2 * j + 2, :],
                            start=(j == 0), stop=(j == KC // 2 - 1),
                            perf_mode=DR,
                        )
                    if mf % 2 == 0:
                        nc.scalar.activation(h_sb[:, mf, :], acc[:], AF.Relu)
                    else:
                        nc.vector.tensor_scalar_max(h_sb[:, mf, :], acc[:],
                                                    0.0)
                return h_sb

            def ffn_w2(blk, h_sb):
                for mb_i in range(MB):
                    accm = ps.tile([P, TB], F32, tag="gemm")
                    for kf in range(FFC):
                        nc.tensor.matmul(
                            accm[:],
                            w2[:, kf, mb_i * P : (mb_i + 1) * P],
                            h_sb[:, kf, :],
                            start=(kf == 0), stop=(kf == FFC - 1),
                        )
                    xs = x_t[:, mb_i, blk * TB : (blk + 1) * TB]
                    nc.vector.tensor_tensor(xs, accm[:], xs, ALU.add)

            def ffn_stats(blk):
                stats_block(tp,
                            lambda k: x_t[:, k, blk * TB : (blk + 1) * TB],
                            s1_ps, s2_ps, blk, blk == 0, blk == NBLK - 1)

            hprev = None
            for blk in range(NBLK):
                apply_ln_upto(tp, blk + 1)
                h_sb = ffn_w1(blk)
                if hprev is not None:
                    ffn_stats(blk - 1)
                ffn_w2(blk, h_sb)
                hprev = h_sb
            ffn_stats(NBLK - 1)
            a_sb, c_sb = ln_chain(s1_ps, s2_ps, NBLK)
            pending_ln[0] = (
                a_sb, c_sb,
                lambda m, blk: x_t[:, m, blk * TB : (blk + 1) * TB],
                NBLK, set())

    # final LN (lnf): skipped. ln3 output has exact zero mean and variance
    # v/(v+eps); applying lnf on top changes values by O(eps)=1e-6, far below
    # the kernel's bf16-level error floor.
    # ---------------- pooling + feature head + cosine ----------------
    with ExitStack() as sctx:
        fp = sctx.enter_context(tc.tile_pool(name="fin", bufs=1))
        fwq = fp.tile([P, KC, F], F32R)
        nc.sync.dma_start(fwq[:], tens["feat_wq"][:])
        fwc = fp.tile([P, KC, F], F32R)
        nc.sync.dma_start(fwc[:], tens["feat_wc"][:])
        NF = F // P  # 2
        NP = BQ * BCC  # 32

        for blk in range(NBLK):
            apply_ln_upto(fp, blk)
            for k in range(KC):
                nc.vector.tensor_reduce(
                    cp[:, k, blk * BCC : (blk + 1) * BCC],
                    x_t[:, k, blk * TB : (blk + 1) * TB]
                    .rearrange("p (c t) -> p c t", c=BCC)[:, :, 1:],
                    mybir.AxisListType.X, ALU.add,
                )
        nc.vector.tensor_scalar_mul(qp[:], qp[:], 1.0 / (TQ - 1))
        nc.vector.tensor_scalar_mul(cp[:], cp[:], 1.0 / (TC - 1))

        qf = fp.tile([P, NF, BQ], F32R)
        cf = fp.tile([P, NF, NP], F32R)
        for fb in range(NF):
            accq = ps.tile([P, TB], F32, tag="gemm")
            accc = ps.tile([P, TB], F32, tag="gemm")
            for k in range(KC):
                nc.tensor.matmul(accq[:, :BQ],
                                 r(fwq[:, k, fb * P : (fb + 1) * P]),
                                 r(qp[:, k, :]),
                                 start=(k == 0), stop=(k == KC - 1))
                nc.tensor.matmul(accc[:, :NP],
                                 r(fwc[:, k, fb * P : (fb + 1) * P]),
                                 r(cp[:, k, :]),
                                 start=(k == 0), stop=(k == KC - 1))
            nc.scalar.copy(qf[:, fb, :], accq[:, :BQ])
            nc.scalar.copy(cf[:, fb, :], accc[:, :NP])

        qsq = fp.tile([P, NF, BQ], F32R)
        csq = fp.tile([P, NF, NP], F32R)
        z = fp.tile([P, NF, NP], F32R)
        qq_ps = ps_attn.tile([8, TB], F32, tag="attn")
        cc_ps = ps_attn.tile([8, TB], F32, tag="attn")
        raw_ps = ps_attn.tile([8, TB], F32, tag="attn")
        for fb in range(NF):
            nc.scalar.activation(qsq[:, fb, :], qf[:, fb, :], AF.Square)
            nc.scalar.activation(csq[:, fb, :], cf[:, fb, :], AF.Square)
            nc.vector.tensor_tensor(
                z[:, fb, :].rearrange("p (e c) -> p e c", e=BQ),
                cf[:, fb, :].rearrange("p (e c) -> p e c", e=BQ),
                qf[:, fb, :, None].to_broadcast((P, BQ, BCC)),
                ALU.mult,
            )
            nc.tensor.matmul(qq_ps[:, :BQ], r(colsel[:, 0, :]), r(qsq[:, fb, :]),
                             start=(fb == 0), stop=(fb == NF - 1))
            nc.tensor.matmul(cc_ps[:, :NP], r(colsel[:, 0, :]), r(csq[:, fb, :]),
                             start=(fb == 0), stop=(fb == NF - 1))
            nc.tensor.matmul(raw_ps[:, :NP], r(colsel[:, 0, :]), r(z[:, fb, :]),
                             start=(fb == 0), stop=(fb == NF - 1))

        def rnorm(src_ps, n, tag):
            t1 = fp.tile([1, NP], F32, tag=f"{tag}1")
            nc.vector.tensor_scalar_max(t1[:, :n], src_ps[0:1, :n], 1e-12)
            t2 = fp.tile([1, NP], F32, tag=f"{tag}2")
            nc.scalar.activation(t2[:, :n], t1[:, :n], AF.Sqrt, bias=0.0)
            t3 = fp.tile([1, NP], F32, tag=f"{tag}3")
            nc.vector.reciprocal(t3[:, :n], t2[:, :n])
            return t3

        rq = rnorm(qq_ps, BQ, "rq")
        rc = rnorm(cc_ps, NP, "rc")
        o1 = fp.tile([1, NP], F32)
        nc.vector.tensor_tensor(o1[:], raw_ps[0:1, :NP], rc[:, :NP], ALU.mult)
        o2 = fp.tile([1, NP], F32)
        nc.vector.tensor_tensor(
            o2[:].rearrange("p (e c) -> p e c", e=BQ),
            o1[:].rearrange("p (e c) -> p e c", e=BQ),
            rq[:, :BQ, None].to_broadcast((1, BQ, BCC)),
            ALU.mult,
        )
        nc.sync.dma_start(tens["out"][:], o2[:])


# ================= host side =================

def _prep_inputs(inputs):
    """Build the per-core DRAM input maps from the full problem inputs."""
    import ml_dtypes

    f32 = np.float32
    bf16 = ml_dtypes.bfloat16
    f8 = ml_dtypes.float8_e4m3fn
    gi = {k: np.asarray(v, f32) for k, v in inputs.items()}

    def to_pkm(w2d, m):
        """[D, m] -> [P, D//P, m] with w[p, k, :] = w2d[k*P + p]."""
        return np.ascontiguousarray(
            w2d.reshape(KC, P, m).transpose(1, 0, 2))

    shared = {}
    q = gi["q"]  # [8, 128, 512]
    qfm = q.reshape(TQALL, D).T.reshape(KC, P, TQALL).transpose(1, 0, 2)
    qfm = np.ascontiguousarray(qfm)
    shared["q_bf"] = qfm.astype(bf16)
    shared["q8"] = (qfm * SX).astype(f8)
    for l in range(L):
        for pfx in ("sa", "ca"):
            for wn in ("wq", "wk", "wv"):
                w = gi[f"{pfx}_{wn}"][l].reshape(D, D)
                shared[f"{pfx}_{wn}8_{l}"] = (to_pkm(w, D) * SW).astype(f8)
            wo = gi[f"{pfx}_wo"][l]  # [N, D, H]
            wo2 = wo.transpose(0, 2, 1).reshape(D, D)  # rows (n,h), cols d
            shared[f"{pfx}_wo8_{l}"] = (to_pkm(wo2, D) * SWO).astype(f8)
            for bn in ("bq", "bk"):
                b = gi[f"{pfx}_{bn}"][l].reshape(D)
                shared[f"{pfx}_{bn}_{l}"] = np.ascontiguousarray(
                    b.reshape(MB, P).T)
            # fold V bias through wo:  bo' = bo + wo.T @ bv
            bv = gi[f"{pfx}_bv"][l].reshape(D)   # (n, h) flattened
            bo = gi[f"{pfx}_bo"][l].reshape(D)
            bo_f = bo + wo2.T @ bv
            shared[f"{pfx}_bo_{l}"] = np.ascontiguousarray(
                bo_f.reshape(MB, P).T.astype(f32))
        shared[f"ffn_w1_{l}"] = (to_pkm(gi["ffn_w1"][l], FF) * SW).astype(f8)
        shared[f"ffn_w2_{l}"] = np.ascontiguousarray(
            gi["ffn_w2"][l].reshape(FFC, P, D).transpose(1, 0, 2)).astype(bf16)
    shared["feat_wq"] = np.ascontiguousarray(
        gi["feat_wq"].reshape(KC, P, F).transpose(1, 0, 2))
    shared["feat_wc"] = np.ascontiguousarray(
        gi["feat_wc"].reshape(KC, P, F).transpose(1, 0, 2))

    colsel = np.zeros((P, 8, 8), f32)
    for j in range(8):
        colsel[:, j, j] = 1.0
    rowsel = np.zeros((8, 8, P), f32)
    for j in range(8):
        rowsel[j, j, :] = 1.0
    def pos_of(n):
        return n // 2 if n % 2 == 0 else 4 + n // 2
    selpair = np.zeros((8, 4, P), f32)
    for hp in range(4):
        selpair[pos_of(2 * hp), hp, :H] = 1.0
        selpair[pos_of(2 * hp + 1), hp, H:] = 1.0
    shared["colsel"] = colsel
    shared["colsel_bf"] = (colsel / D).astype(bf16)
    shared["densel_bf"] = colsel.astype(bf16)
    shared["rowsel_bf"] = rowsel.astype(bf16)
    shared["selpair_bf"] = selpair.astype(bf16)

    c = gi["c"]  # [32, 128, 512]
    in_maps = []
    for cc in range(NCORES):
        m = dict(shared)
        sl = c[cc * BCC : (cc + 1) * BCC].reshape(T1, D)
        x0 = sl.T.reshape(KC, P, T1).transpose(1, 0, 2)
        m["x0"] = np.ascontiguousarray(x0).astype(bf16)
        in_maps.append(m)
    return in_maps


def kernel(**inputs):
    global _BUILT
    from concourse import bass_utils

    if _BUILT is None:
        _BUILT = build_program()
    nc = _BUILT
    in_maps = _prep_inputs(inputs)
    res = bass_utils.run_bass_kernel_spmd(nc, in_maps, list(range(NCORES)))
    outs = [res.results[i]["out"].reshape(BQ, BCC) for i in range(NCORES)]
    return np.concatenate(outs, axis=1).astype(np.float32)

